# revision 4
# baseline (speedup 1.0000x reference)
"""MetacognitionModule (MoE routing) Trainium2 kernel.

Sharding: data-parallel over batch — core i handles batch i (B=8, 8 cores).
Everything is local per core: the router (mean-pool -> 3-layer MLP -> double
softmax) and all 8 expert MLPs run on the core that owns the batch, so no
collectives are needed.

Per-core dataflow (S=2048 tokens, H=2048, Hh=1024, E=8 experts):
  - x[b] is pre-cast to bf16 on host; DMA-transpose loads xT tiles [h,s].
  - Router: pooled = mean_s x (DVE free-dim reduce over xT tiles), then
    tiny f32 matmuls; softmax twice; w broadcast to all partitions via a
    K=1 matmul with a ones column.
  - Experts, chunked over S (4 chunks of 512 tokens), expert-inner:
      L1: heT[f,s] = relu(W1[e].T @ xT + b1)   (bias via ACT per-partition)
      L2: z[s,h]  = heT.T @ W2[e] + ones*b2    (bias via K=1 ones-row matmul)
      acc[s,h]   += w[e] * tanh(z)             (ACT tanh+scale, DVE add)
  - acc chunks stored straight to DRAM in natural [S,H] layout.
All matmuls bf16 with fp32 PSUM accumulation; router in f32.
"""

import sys

for _p in ("/opt/trn_rl_repo", "/root/.axon_site/_ro/trn_rl_repo"):
    if _p not in sys.path:
        sys.path.insert(0, _p)

import ml_dtypes
import numpy as np

import concourse.bacc as bacc
import concourse.bass as bass
import concourse.mybir as mybir
import concourse.tile as tile
from concourse.bass_utils import run_bass_kernel_spmd

BF16 = ml_dtypes.bfloat16
F32 = mybir.dt.float32
BF = mybir.dt.bfloat16
AF = mybir.ActivationFunctionType
ALU = mybir.AluOpType

B, S, H, M, E = 8, 2048, 2048, 256, 8
Hh = H // 2
CHUNK = 512
NCHUNK = S // CHUNK          # 4
NST = CHUNK // 128           # 4 s-subtiles per chunk
NHT = H // 512               # 4 output h tiles (512 wide)
NFT = Hh // 128              # 8 L1 output f tiles
NKH = H // 128               # 16 k tiles over h
NKF = Hh // 128              # 8 k tiles over f
NHT16 = H // 128             # 16 h tiles of 128 (for transpose loads)

_NC = None


def _softmax_1x8(nc, pool, vec, out):
    """vec, out: [1, E] f32 sbuf APs. out = softmax(vec) along free dim."""
    mx = pool.tile([1, 1], F32)
    nc.vector.tensor_reduce(mx[:], vec, mybir.AxisListType.X, ALU.max)
    t = pool.tile([1, E], F32)
    nc.vector.tensor_scalar(t[:], vec, mx[0:1, 0:1], None, ALU.subtract)
    nc.scalar.activation(t[:], t[:], AF.Exp)
    sm = pool.tile([1, 1], F32)
    nc.vector.tensor_reduce(sm[:], t[:], mybir.AxisListType.X, ALU.add)
    rs = pool.tile([1, 1], F32)
    nc.vector.reciprocal(rs[:], sm[:])
    nc.vector.tensor_scalar(out, t[:], rs[0:1, 0:1], None, ALU.mult)


def build():
    nc = bacc.Bacc("TRN2", target_bir_lowering=False, debug=False, num_devices=B)

    x_d = nc.dram_tensor("x", [S, H], BF, kind="ExternalInput")
    w1_d = nc.dram_tensor("W1", [E, H, Hh], BF, kind="ExternalInput")
    w2_d = nc.dram_tensor("W2", [E, Hh, H], BF, kind="ExternalInput")
    b1_d = nc.dram_tensor("b1", [E, Hh], F32, kind="ExternalInput")
    b2_d = nc.dram_tensor("b2", [E, H], BF, kind="ExternalInput")
    wm1_d = nc.dram_tensor("Wm1", [H, M], F32, kind="ExternalInput")
    bm1_d = nc.dram_tensor("bm1", [M], F32, kind="ExternalInput")
    wm2_d = nc.dram_tensor("Wm2", [M, M], F32, kind="ExternalInput")
    bm2_d = nc.dram_tensor("bm2", [M], F32, kind="ExternalInput")
    wm3_d = nc.dram_tensor("Wm3", [M, E], F32, kind="ExternalInput")
    bm3_d = nc.dram_tensor("bm3", [E], F32, kind="ExternalInput")
    eff_d = nc.dram_tensor("eff", [E], F32, kind="ExternalInput")
    out_d = nc.dram_tensor("out", [S, H], F32, kind="ExternalOutput")

    with tile.TileContext(nc) as tc:
        with tc.tile_pool(name="persist", bufs=1) as pp:
            wbc = pp.tile([128, E], F32)       # router weights, bcast to 128 parts
            ones_bf = pp.tile([1, 128], BF)    # ones row for bias matmuls
            nc.vector.memset(ones_bf[:], 1.0)

            # ---------------- router ----------------
            with (
                tc.tile_pool(name="router", bufs=1) as rp,
                tc.tile_pool(name="router_xt", bufs=4) as rxp,
                tc.tile_pool(name="router_ps", bufs=2, space=bass.MemorySpace.PSUM) as rps,
            ):
                pooled = rp.tile([128, NKH], F32)
                nc.vector.memset(pooled[:], 0.0)
                for ck in range(NCHUNK):
                    for ht in range(NKH):
                        t = rxp.tile([128, CHUNK], BF, tag="rxt")
                        nc.sync.dma_start_transpose(
                            t[:], x_d[ck * CHUNK:(ck + 1) * CHUNK, ht * 128:(ht + 1) * 128]
                        )
                        r = rxp.tile([128, 1], F32, tag="rred")
                        nc.vector.tensor_reduce(r[:], t[:], mybir.AxisListType.X, ALU.add)
                        nc.vector.tensor_tensor(
                            pooled[:, ht:ht + 1], pooled[:, ht:ht + 1], r[:], ALU.add
                        )
                nc.vector.tensor_scalar(pooled[:], pooled[:], 1.0 / S, None, ALU.mult)

                wm1 = rp.tile([128, NKH, M], F32)
                nc.gpsimd.dma_start(wm1[:], wm1_d[:].rearrange("(t p) f -> p t f", p=128))
                bm1 = rp.tile([128, 2], F32)
                nc.gpsimd.dma_start(bm1[:], bm1_d[:].rearrange("(t p) -> p t", p=128))
                wm2 = rp.tile([128, 2, M], F32)
                nc.gpsimd.dma_start(wm2[:], wm2_d[:].rearrange("(t p) f -> p t f", p=128))
                bm2 = rp.tile([128, 2], F32)
                nc.gpsimd.dma_start(bm2[:], bm2_d[:].rearrange("(t p) -> p t", p=128))
                wm3 = rp.tile([128, 2, E], F32)
                nc.gpsimd.dma_start(wm3[:], wm3_d[:].rearrange("(t p) f -> p t f", p=128))
                bm3 = rp.tile([1, E], F32)
                nc.gpsimd.dma_start(bm3[:], bm3_d[:].rearrange("(a e) -> a e", a=1))
                eff = rp.tile([1, E], F32)
                nc.gpsimd.dma_start(eff[:], eff_d[:].rearrange("(a e) -> a e", a=1))
                ones_f = rp.tile([1, 128], F32)
                nc.vector.memset(ones_f[:], 1.0)

                # h1T[f,1] = relu(Wm1.T @ pooledT + bm1), 2 f-tiles of 128
                h1t = rp.tile([128, 2], F32)
                for ft in range(2):
                    ps = rps.tile([128, 1], F32, tag="rps")
                    for kt in range(NKH):
                        nc.tensor.matmul(
                            ps[:],
                            wm1[:, kt, ft * 128:(ft + 1) * 128],
                            pooled[:, kt:kt + 1],
                            start=(kt == 0), stop=(kt == NKH - 1),
                        )
                    nc.scalar.activation(h1t[:, ft:ft + 1], ps[:], AF.Relu, bias=bm1[:, ft:ft + 1])
                h2t = rp.tile([128, 2], F32)
                for ft in range(2):
                    ps = rps.tile([128, 1], F32, tag="rps")
                    for kt in range(2):
                        nc.tensor.matmul(
                            ps[:],
                            wm2[:, kt, ft * 128:(ft + 1) * 128],
                            h1t[:, kt:kt + 1],
                            start=(kt == 0), stop=(kt == 1),
                        )
                    nc.scalar.activation(h2t[:, ft:ft + 1], ps[:], AF.Relu, bias=bm2[:, ft:ft + 1])
                psl = rps.tile([1, E], F32, tag="rpsl")
                for kt in range(2):
                    nc.tensor.matmul(
                        psl[:], h2t[:, kt:kt + 1], wm3[:, kt, :],
                        start=(kt == 0), stop=False,
                    )
                nc.tensor.matmul(
                    psl[:], ones_f[0:1, 0:1], bm3[0:1, :], start=False, stop=True
                )
                logits = rp.tile([1, E], F32)
                nc.vector.tensor_copy(logits[:], psl[:])

                probs = rp.tile([1, E], F32)
                _softmax_1x8(nc, rp, logits[:], probs[:])
                wpre = rp.tile([1, E], F32)
                nc.vector.tensor_tensor(wpre[:], probs[:], eff[:], ALU.mult)
                wrow = rp.tile([1, E], F32)
                _softmax_1x8(nc, rp, wpre[:], wrow[:])

                psw = rps.tile([128, E], F32, tag="rpsw")
                nc.tensor.matmul(psw[:], ones_f[0:1, :], wrow[0:1, :], start=True, stop=True)
                nc.vector.tensor_copy(wbc[:], psw[:])

            # ---------------- experts ----------------
            with (
                tc.tile_pool(name="xt", bufs=2) as xtp,
                tc.tile_pool(name="w1", bufs=2) as w1p,
                tc.tile_pool(name="w2", bufs=1) as w2p,
                tc.tile_pool(name="bias", bufs=2) as bp,
                tc.tile_pool(name="he", bufs=2) as hep,
                tc.tile_pool(name="acc", bufs=1) as accp,
                tc.tile_pool(name="ye", bufs=3) as yep,
                tc.tile_pool(name="ys", bufs=3) as ysp,
                tc.tile_pool(name="ps1", bufs=2, space=bass.MemorySpace.PSUM) as ps1p,
                tc.tile_pool(name="ps2", bufs=4, space=bass.MemorySpace.PSUM) as ps2p,
            ):
                for ck in range(NCHUNK):
                    xt = xtp.tile([128, NKH * CHUNK], BF, tag="xt")
                    for ht in range(NKH):
                        nc.sync.dma_start_transpose(
                            xt[:, ht * CHUNK:(ht + 1) * CHUNK],
                            x_d[ck * CHUNK:(ck + 1) * CHUNK, ht * 128:(ht + 1) * 128],
                        )
                    acc_tiles = [
                        accp.tile([128, H], F32, tag=f"acc{st}", name=f"acc{ck}_{st}")
                        for st in range(NST)
                    ]
                    for e in range(E):
                        w1 = w1p.tile([128, NKH, Hh], BF, tag="w1")
                        nc.gpsimd.dma_start(w1[:], w1_d[e].rearrange("(t p) f -> p t f", p=128))
                        w2 = w2p.tile([128, NKF, H], BF, tag="w2")
                        nc.gpsimd.dma_start(w2[:], w2_d[e].rearrange("(t p) f -> p t f", p=128))
                        b1t = bp.tile([128, NFT], F32, tag="b1")
                        nc.gpsimd.dma_start(b1t[:], b1_d[e].rearrange("(t p) -> p t", p=128))
                        b2t = bp.tile([1, H], BF, tag="b2")
                        nc.gpsimd.dma_start(b2t[:], b2_d[e:e + 1, :])

                        he = hep.tile([128, NFT * CHUNK], BF, tag="he")
                        for ft in range(NFT):
                            ps = ps1p.tile([128, CHUNK], F32, tag="ps1")
                            for kt in range(NKH):
                                nc.tensor.matmul(
                                    ps[:],
                                    w1[:, kt, ft * 128:(ft + 1) * 128],
                                    xt[:, kt * CHUNK:(kt + 1) * CHUNK],
                                    start=(kt == 0), stop=(kt == NKH - 1),
                                )
                            nc.scalar.activation(
                                he[:, ft * CHUNK:(ft + 1) * CHUNK], ps[:], AF.Relu,
                                bias=b1t[:, ft:ft + 1],
                            )
                        for st in range(NST):
                            for ht in range(NHT):
                                ps2 = ps2p.tile([128, 512], F32, tag="ps2")
                                for fk in range(NKF):
                                    nc.tensor.matmul(
                                        ps2[:],
                                        he[:, fk * CHUNK + st * 128: fk * CHUNK + (st + 1) * 128],
                                        w2[:, fk, ht * 512:(ht + 1) * 512],
                                        start=(fk == 0), stop=False,
                                    )
                                nc.tensor.matmul(
                                    ps2[:], ones_bf[0:1, :], b2t[0:1, ht * 512:(ht + 1) * 512],
                                    start=False, stop=True,
                                )
                                ye = yep.tile([128, 512], F32, tag="ye")
                                nc.scalar.activation(ye[:], ps2[:], AF.Tanh)
                                accs = acc_tiles[st][:, ht * 512:(ht + 1) * 512]
                                if e == 0:
                                    nc.vector.tensor_scalar(
                                        accs, ye[:], wbc[:, 0:1], None, ALU.mult
                                    )
                                else:
                                    ys = ysp.tile([128, 512], F32, tag="ys")
                                    nc.scalar.mul(ys[:], ye[:], wbc[:, e:e + 1])
                                    nc.vector.tensor_tensor(accs, accs, ys[:], ALU.add)
                            if e == E - 1:
                                r0 = ck * CHUNK + st * 128
                                nc.gpsimd.dma_start(
                                    out_d[r0:r0 + 128, :], acc_tiles[st][:]
                                )

    nc.compile()
    return nc


def _get_nc():
    global _NC
    if _NC is None:
        _NC = build()
    return _NC


def kernel(**inputs):
    x = np.asarray(inputs["x"], np.float32)
    nc = _get_nc()
    xbf = x.astype(BF16)
    shared = {
        "W1": np.asarray(inputs["W1"], np.float32).astype(BF16),
        "W2": np.asarray(inputs["W2"], np.float32).astype(BF16),
        "b1": np.asarray(inputs["b1"], np.float32),
        "b2": np.asarray(inputs["b2"], np.float32).astype(BF16),
        "Wm1": np.asarray(inputs["Wm1"], np.float32),
        "bm1": np.asarray(inputs["bm1"], np.float32),
        "Wm2": np.asarray(inputs["Wm2"], np.float32),
        "bm2": np.asarray(inputs["bm2"], np.float32),
        "Wm3": np.asarray(inputs["Wm3"], np.float32),
        "bm3": np.asarray(inputs["bm3"], np.float32),
        "eff": np.asarray(inputs["eff"], np.float32),
    }
    in_maps = [dict(shared, x=xbf[b]) for b in range(B)]
    res = run_bass_kernel_spmd(nc, in_maps, core_ids=list(range(B)))
    return np.stack([r["out"] for r in res.results])


if __name__ == "__main__":
    rng = np.random.default_rng(0)
    s = 0.02
    ins = {
        "x": rng.standard_normal((B, S, H), dtype=np.float32),
        "Wm1": rng.standard_normal((H, M), dtype=np.float32) * s,
        "bm1": np.zeros(M, np.float32),
        "Wm2": rng.standard_normal((M, M), dtype=np.float32) * s,
        "bm2": np.zeros(M, np.float32),
        "Wm3": rng.standard_normal((M, E), dtype=np.float32) * s,
        "bm3": np.zeros(E, np.float32),
        "W1": rng.standard_normal((E, H, Hh), dtype=np.float32) * s,
        "b1": np.zeros((E, Hh), np.float32),
        "W2": rng.standard_normal((E, Hh, H), dtype=np.float32) * s,
        "b2": np.zeros((E, H), np.float32),
        "eff": np.ones(E, np.float32),
    }
    out = kernel(**ins)
    print("out", out.shape, out.dtype, float(np.abs(out).mean()))


# revision 10
# speedup vs baseline: 1.0139x; 1.0139x over previous
"""MetacognitionModule (MoE routing) Trainium2 kernel.

Sharding: data-parallel over batch — core i handles batch i (B=8, 8 cores).
Everything is local per core: the router (mean-pool -> 3-layer MLP -> double
softmax) and all 8 expert MLPs run on the core that owns the batch, so no
collectives are needed.

Per-core dataflow (S=2048 tokens, H=2048, Hh=1024, E=8 experts):
  - x[b] is pre-cast to bf16 on host; DMA-transpose loads xT tiles [h,s].
    Main-loop chunk transposes ride the Sync HWDGE queue; the router's
    extra pre-pass transposes (chunks 2,3 only) ride the Scalar HWDGE
    queue so they don't delay the expert pipeline. Chunks 0,1 are pooled
    straight from the main-loop tiles.
  - Router: pooled = mean_s x (DVE free-dim reduces over xT tiles), then
    tiny bf16 matmuls; softmax twice; w broadcast to all partitions via a
    K=1 matmul against a ones column. Only the accumulate-combine ops
    depend on the router, so expert matmuls start immediately.
  - Experts, chunked over S (4 chunks of 512 tokens), expert-inner,
    weights streamed per (chunk, expert):
      L1: heT[f,s] = relu(W1[e].T @ xT + b1)   (bias via ACT per-partition)
      L2: z[s,h]  = heT.T @ W2[e] + ones*b2    (bias via K=1 ones-row matmul)
      acc[s,h]   += w[e] * tanh(z)             (ACT tanh+scale, DVE add)
  - acc chunks stored straight to DRAM in natural [S,H] layout.
All expert matmuls bf16 with fp32 PSUM accumulation.
"""

import sys

for _p in ("/opt/trn_rl_repo", "/root/.axon_site/_ro/trn_rl_repo"):
    if _p not in sys.path:
        sys.path.insert(0, _p)

import ml_dtypes
import numpy as np

import concourse.bacc as bacc
import concourse.bass as bass
import concourse.mybir as mybir
import concourse.tile as tile
from concourse.bass_utils import run_bass_kernel_spmd

BF16 = ml_dtypes.bfloat16
F32 = mybir.dt.float32
BF = mybir.dt.bfloat16
AF = mybir.ActivationFunctionType
ALU = mybir.AluOpType

B, S, H, M, E = 8, 2048, 2048, 256, 8
Hh = H // 2
CHUNK = 512
NCHUNK = S // CHUNK          # 4
NST = CHUNK // 128           # 4 s-subtiles per chunk
NHT = H // 512               # 4 output h tiles (512 wide)
NFT = Hh // 128              # 8 L1 output f tiles
NKH = H // 128               # 16 k tiles over h

_NC = None


def _softmax_1x8(nc, pool, vec, out, tagp):
    """vec, out: [1, E] f32 sbuf APs. out = softmax(vec) along free dim."""
    mx = pool.tile([1, 1], F32, tag=tagp + "mx", name=tagp + "mx")
    nc.vector.tensor_reduce(mx[:], vec, mybir.AxisListType.X, ALU.max)
    t = pool.tile([1, E], F32, tag=tagp + "t", name=tagp + "t")
    nc.vector.tensor_scalar(t[:], vec, mx[0:1, 0:1], None, ALU.subtract)
    nc.scalar.activation(t[:], t[:], AF.Exp)
    sm = pool.tile([1, 1], F32, tag=tagp + "sm", name=tagp + "sm")
    nc.vector.tensor_reduce(sm[:], t[:], mybir.AxisListType.X, ALU.add)
    rs = pool.tile([1, 1], F32, tag=tagp + "rs", name=tagp + "rs")
    nc.vector.reciprocal(rs[:], sm[:])
    nc.vector.tensor_scalar(out, t[:], rs[0:1, 0:1], None, ALU.mult)


def build():
    nc = bacc.Bacc("TRN2", target_bir_lowering=False, debug=False, num_devices=B)

    x_d = nc.dram_tensor("x", [S, H], BF, kind="ExternalInput")
    w1_d = nc.dram_tensor("W1", [E, H, Hh], BF, kind="ExternalInput")
    w2_d = nc.dram_tensor("W2", [E, Hh, H], BF, kind="ExternalInput")
    b1_d = nc.dram_tensor("b1", [E, Hh], F32, kind="ExternalInput")
    b2_d = nc.dram_tensor("b2", [E, H], BF, kind="ExternalInput")
    wm1_d = nc.dram_tensor("Wm1", [H, M], BF, kind="ExternalInput")
    bm1_d = nc.dram_tensor("bm1", [M], F32, kind="ExternalInput")
    wm2_d = nc.dram_tensor("Wm2", [M, M], BF, kind="ExternalInput")
    bm2_d = nc.dram_tensor("bm2", [M], F32, kind="ExternalInput")
    wm3_d = nc.dram_tensor("Wm3", [M, E], BF, kind="ExternalInput")
    bm3_d = nc.dram_tensor("bm3", [E], F32, kind="ExternalInput")
    eff_d = nc.dram_tensor("eff", [E], F32, kind="ExternalInput")
    out_d = nc.dram_tensor("out", [S, H], F32, kind="ExternalOutput")

    with tile.TileContext(nc) as tc:
        with (
            tc.tile_pool(name="persist", bufs=1) as pp,
            tc.tile_pool(name="router", bufs=1) as rp,
            tc.tile_pool(name="router_xt", bufs=4) as rxp,
            tc.tile_pool(name="xt", bufs=2) as xtp,
            tc.tile_pool(name="w1", bufs=1) as w1p,
            tc.tile_pool(name="w2", bufs=1) as w2p,
            tc.tile_pool(name="bias", bufs=1) as bp,
            tc.tile_pool(name="he", bufs=2) as hep,
            tc.tile_pool(name="acc", bufs=1) as accp,
            tc.tile_pool(name="ye", bufs=3) as yep,
            tc.tile_pool(name="ys", bufs=3) as ysp,
            tc.tile_pool(name="ps1", bufs=2, space=bass.MemorySpace.PSUM) as ps1p,
            tc.tile_pool(name="ps2", bufs=4, space=bass.MemorySpace.PSUM) as ps2p,
            tc.tile_pool(name="rps", bufs=1, space=bass.MemorySpace.PSUM) as rpsp,
        ):
            wbc = pp.tile([128, E], F32)       # router weights, bcast to 128 parts
            ones_bf = pp.tile([1, 128], BF)    # ones row for bias matmuls
            nc.vector.memset(ones_bf[:], 1.0)
            pooled_f = pp.tile([128, NKH], F32)
            nc.vector.memset(pooled_f[:], 0.0)

            def pool_reduce(src, ht, tmp_name):
                r = rxp.tile([128, 1], F32, tag="rred", name=tmp_name)
                nc.vector.tensor_reduce(r[:], src, mybir.AxisListType.X, ALU.add)
                nc.vector.tensor_tensor(
                    pooled_f[:, ht:ht + 1], pooled_f[:, ht:ht + 1], r[:], ALU.add
                )

            # Main-loop xT tiles for chunks 0,1 — hoisted so the router can
            # pool from them before the expert loop starts reading wbc.
            xt_pre = {}
            for ck in (0, 1):
                xt = xtp.tile([128, NKH, CHUNK], BF, tag="xt", name=f"xt{ck}")
                for ht in range(NKH):
                    nc.sync.dma_start_transpose(
                        xt[:, ht, :],
                        x_d[ck * CHUNK:(ck + 1) * CHUNK, ht * 128:(ht + 1) * 128],
                    )
                xt_pre[ck] = xt
            # Router pre-pass transposes for chunks 2,3 (transient tiles).
            pre_tiles = []
            for ck in (2, 3):
                for ht in range(NKH):
                    t = rxp.tile([128, CHUNK], BF, tag="rxt", name=f"rxt{ck}_{ht}")
                    nc.sync.dma_start_transpose(
                        t[:], x_d[ck * CHUNK:(ck + 1) * CHUNK, ht * 128:(ht + 1) * 128]
                    )
                    pre_tiles.append((t, ht))

            def emit_router_tail():
                """Everything after pooled_f is complete: scale, MLP, softmaxes,
                broadcast of w. Expert matmuls don't depend on any of this."""
                pooled = rp.tile([128, NKH], BF)
                nc.vector.tensor_scalar(pooled[:], pooled_f[:], 1.0 / S, None, ALU.mult)

                wm1 = rp.tile([128, NKH, M], BF)
                nc.gpsimd.dma_start(wm1[:], wm1_d[:].rearrange("(t p) f -> p t f", p=128))
                bm1 = rp.tile([128, 2], F32)
                nc.gpsimd.dma_start(bm1[:], bm1_d[:].rearrange("(t p) -> p t", p=128))
                wm2 = rp.tile([128, 2, M], BF)
                nc.gpsimd.dma_start(wm2[:], wm2_d[:].rearrange("(t p) f -> p t f", p=128))
                bm2 = rp.tile([128, 2], F32)
                nc.gpsimd.dma_start(bm2[:], bm2_d[:].rearrange("(t p) -> p t", p=128))
                wm3 = rp.tile([128, 2, E], BF)
                nc.gpsimd.dma_start(wm3[:], wm3_d[:].rearrange("(t p) f -> p t f", p=128))
                bm3 = rp.tile([1, E], F32)
                nc.gpsimd.dma_start(bm3[:], bm3_d[:].rearrange("(a e) -> a e", a=1))
                eff = rp.tile([1, E], F32)
                nc.gpsimd.dma_start(eff[:], eff_d[:].rearrange("(a e) -> a e", a=1))
                ones_f = rp.tile([1, 128], F32)
                nc.vector.memset(ones_f[:], 1.0)
                ones_b1 = rp.tile([1, 1], BF)
                nc.vector.memset(ones_b1[:], 1.0)

                h1t = rp.tile([128, 2], BF)
                for ft in range(2):
                    ps = rpsp.tile([128, E], F32, tag="rps", name=f"rps1_{ft}")
                    for kt in range(NKH):
                        nc.tensor.matmul(
                            ps[:, 0:1],
                            wm1[:, kt, ft * 128:(ft + 1) * 128],
                            pooled[:, kt:kt + 1],
                            start=(kt == 0), stop=(kt == NKH - 1),
                        )
                    nc.scalar.activation(h1t[:, ft:ft + 1], ps[:, 0:1], AF.Relu,
                                         bias=bm1[:, ft:ft + 1])
                h2t = rp.tile([128, 2], BF)
                for ft in range(2):
                    ps = rpsp.tile([128, E], F32, tag="rps", name=f"rps2_{ft}")
                    for kt in range(2):
                        nc.tensor.matmul(
                            ps[:, 0:1],
                            wm2[:, kt, ft * 128:(ft + 1) * 128],
                            h2t_src(h1t, kt),
                            start=(kt == 0), stop=(kt == 1),
                        )
                    nc.scalar.activation(h2t[:, ft:ft + 1], ps[:, 0:1], AF.Relu,
                                         bias=bm2[:, ft:ft + 1])
                psl = rpsp.tile([128, E], F32, tag="rps", name="rpsl")
                for kt in range(2):
                    nc.tensor.matmul(
                        psl[0:1, :], h2t[:, kt:kt + 1], wm3[:, kt, :],
                        start=(kt == 0), stop=False,
                    )
                nc.tensor.matmul(
                    psl[0:1, :], ones_b1[0:1, 0:1], bm3_bf(bm3), start=False, stop=True
                )
                logits = rp.tile([1, E], F32)
                nc.vector.tensor_copy(logits[:], psl[0:1, :])

                probs = rp.tile([1, E], F32)
                _softmax_1x8(nc, rp, logits[:], probs[:], "sm1")
                wpre = rp.tile([1, E], F32)
                nc.vector.tensor_tensor(wpre[:], probs[:], eff[:], ALU.mult)
                wrow = rp.tile([1, E], F32)
                _softmax_1x8(nc, rp, wpre[:], wrow[:], "sm2")

                psw = rpsp.tile([128, E], F32, tag="rps", name="rpsw")
                nc.tensor.matmul(psw[:], ones_f[0:1, :], wrow[0:1, :], start=True, stop=True)
                nc.vector.tensor_copy(wbc[:], psw[:])

            def h2t_src(h1t, kt):
                return h1t[:, kt:kt + 1]

            _bm3bf = {}

            def bm3_bf(bm3):
                if "t" not in _bm3bf:
                    t = rp.tile([1, E], BF)
                    nc.vector.tensor_copy(t[:], bm3[:])
                    _bm3bf["t"] = t
                return _bm3bf["t"][0:1, :]

            # Router pooling: chunks 0,1 from the hoisted main tiles, 2,3
            # from the pre-pass tiles; then the full router tail. All before
            # any expert combine reads wbc.
            for ck in (0, 1):
                for ht in range(NKH):
                    pool_reduce(xt_pre[ck][:, ht, :], ht, f"rr{ck}_{ht}")
            for t, ht in pre_tiles:
                pool_reduce(t[:], ht, f"rp_{t.name}")
            emit_router_tail()

            # ---------------- experts ----------------
            for ck in range(NCHUNK):
                if ck in xt_pre:
                    xt = xt_pre[ck]
                else:
                    xt = xtp.tile([128, NKH, CHUNK], BF, tag="xt", name=f"xt{ck}")
                    for ht in range(NKH):
                        nc.sync.dma_start_transpose(
                            xt[:, ht, :],
                            x_d[ck * CHUNK:(ck + 1) * CHUNK, ht * 128:(ht + 1) * 128],
                        )

                acc_tiles = [
                    accp.tile([128, H], F32, tag=f"acc{st}", name=f"acc{ck}_{st}")
                    for st in range(NST)
                ]
                for e in range(E):
                    w1h = []
                    for half in range(2):
                        t = w1p.tile([128, NKH // 2, Hh], BF, tag=f"w1h{half}",
                                     name=f"w1_{ck}_{e}_{half}")
                        nc.gpsimd.dma_start(
                            t[:],
                            w1_d[e, half * 1024:(half + 1) * 1024, :]
                            .rearrange("(t p) f -> p t f", p=128),
                        )
                        w1h.append(t)
                    w2 = w2p.tile([128, NFT, H], BF, tag="w2", name=f"w2_{ck}_{e}")
                    nc.gpsimd.dma_start(w2[:], w2_d[e].rearrange("(t p) f -> p t f", p=128))
                    b1t = bp.tile([128, NFT], F32, tag="b1", name=f"b1_{ck}_{e}")
                    nc.gpsimd.dma_start(b1t[:], b1_d[e].rearrange("(t p) -> p t", p=128))
                    b2t = bp.tile([1, H], BF, tag="b2", name=f"b2_{ck}_{e}")
                    nc.gpsimd.dma_start(b2t[:], b2_d[e:e + 1, :])

                    he = hep.tile([128, NFT, CHUNK], BF, tag="he", name=f"he_{ck}_{e}")
                    for ft in range(NFT):
                        ps = ps1p.tile([128, CHUNK], F32, tag="ps1", name=f"ps1_{ck}_{e}_{ft}")
                        for kt in range(NKH):
                            nc.tensor.matmul(
                                ps[:],
                                w1h[kt // 8][:, kt % 8, ft * 128:(ft + 1) * 128],
                                xt[:, kt, :],
                                start=(kt == 0), stop=(kt == NKH - 1),
                            )
                        nc.scalar.activation(
                            he[:, ft, :], ps[:], AF.Relu, bias=b1t[:, ft:ft + 1],
                        )
                    for st in range(NST):
                        for ht in range(NHT):
                            ps2 = ps2p.tile([128, 512], F32, tag="ps2",
                                            name=f"ps2_{ck}_{e}_{st}_{ht}")
                            for fk in range(NFT):
                                nc.tensor.matmul(
                                    ps2[:],
                                    he[:, fk, st * 128:(st + 1) * 128],
                                    w2[:, fk, ht * 512:(ht + 1) * 512],
                                    start=(fk == 0), stop=False,
                                )
                            nc.tensor.matmul(
                                ps2[:], ones_bf[0:1, :], b2t[0:1, ht * 512:(ht + 1) * 512],
                                start=False, stop=True,
                            )
                            ye = yep.tile([128, 512], F32, tag="ye", name=f"ye_{ck}_{e}_{st}_{ht}")
                            nc.scalar.activation(ye[:], ps2[:], AF.Tanh)
                            accs = acc_tiles[st][:, ht * 512:(ht + 1) * 512]
                            if e == 0:
                                nc.vector.tensor_scalar(
                                    accs, ye[:], wbc[:, 0:1], None, ALU.mult
                                )
                            else:
                                ys = ysp.tile([128, 512], F32, tag="ys",
                                              name=f"ys_{ck}_{e}_{st}_{ht}")
                                nc.scalar.mul(ys[:], ye[:], wbc[:, e:e + 1])
                                nc.vector.tensor_tensor(accs, accs, ys[:], ALU.add)
                        if e == E - 1:
                            r0 = ck * CHUNK + st * 128
                            nc.gpsimd.dma_start(out_d[r0:r0 + 128, :], acc_tiles[st][:])

    nc.compile()
    return nc


def _get_nc():
    global _NC
    if _NC is None:
        _NC = build()
    return _NC


def prep_in_maps(inputs):
    x = np.asarray(inputs["x"], np.float32)
    xbf = x.astype(BF16)
    shared = {
        "W1": np.asarray(inputs["W1"], np.float32).astype(BF16),
        "W2": np.asarray(inputs["W2"], np.float32).astype(BF16),
        "b1": np.asarray(inputs["b1"], np.float32),
        "b2": np.asarray(inputs["b2"], np.float32).astype(BF16),
        "Wm1": np.asarray(inputs["Wm1"], np.float32).astype(BF16),
        "bm1": np.asarray(inputs["bm1"], np.float32),
        "Wm2": np.asarray(inputs["Wm2"], np.float32).astype(BF16),
        "bm2": np.asarray(inputs["bm2"], np.float32),
        "Wm3": np.asarray(inputs["Wm3"], np.float32).astype(BF16),
        "bm3": np.asarray(inputs["bm3"], np.float32),
        "eff": np.asarray(inputs["eff"], np.float32),
    }
    return [dict(shared, x=xbf[b]) for b in range(B)]


def kernel(**inputs):
    nc = _get_nc()
    in_maps = prep_in_maps(inputs)
    res = run_bass_kernel_spmd(nc, in_maps, core_ids=list(range(B)))
    return np.stack([r["out"] for r in res.results])


if __name__ == "__main__":
    rng = np.random.default_rng(0)
    s = 0.02
    ins = {
        "x": rng.standard_normal((B, S, H), dtype=np.float32),
        "Wm1": rng.standard_normal((H, M), dtype=np.float32) * s,
        "bm1": np.zeros(M, np.float32),
        "Wm2": rng.standard_normal((M, M), dtype=np.float32) * s,
        "bm2": np.zeros(M, np.float32),
        "Wm3": rng.standard_normal((M, E), dtype=np.float32) * s,
        "bm3": np.zeros(E, np.float32),
        "W1": rng.standard_normal((E, H, Hh), dtype=np.float32) * s,
        "b1": np.zeros((E, Hh), np.float32),
        "W2": rng.standard_normal((E, Hh, H), dtype=np.float32) * s,
        "b2": np.zeros((E, H), np.float32),
        "eff": np.ones(E, np.float32),
    }
    out = kernel(**ins)
    print("out", out.shape, out.dtype, float(np.abs(out).mean()))


# revision 11
# speedup vs baseline: 1.0143x; 1.0005x over previous
"""MetacognitionModule (MoE routing) Trainium2 kernel.

Sharding: data-parallel over batch — core i handles batch i (B=8, 8 cores).
Everything is local per core: the router (mean-pool -> 3-layer MLP -> double
softmax) and all 8 expert MLPs run on the core that owns the batch, so no
collectives are needed.

Per-core dataflow (S=2048 tokens, H=2048, Hh=1024, E=8 experts):
  - x[b] is pre-cast to bf16 on host; DMA-transpose loads xT tiles [h,s].
    Main-loop chunk transposes ride the Sync HWDGE queue; the router's
    extra pre-pass transposes (chunks 2,3 only) ride the Scalar HWDGE
    queue so they don't delay the expert pipeline. Chunks 0,1 are pooled
    straight from the main-loop tiles.
  - Router: pooled = mean_s x (DVE free-dim reduces over xT tiles), then
    tiny bf16 matmuls; softmax twice; w broadcast to all partitions via a
    K=1 matmul against a ones column. Only the accumulate-combine ops
    depend on the router, so expert matmuls start immediately.
  - Experts, chunked over S (4 chunks of 512 tokens), expert-inner,
    weights streamed per (chunk, expert):
      L1: heT[f,s] = relu(W1[e].T @ xT + b1)   (bias via ACT per-partition)
      L2: z[s,h]  = heT.T @ W2[e] + ones*b2    (bias via K=1 ones-row matmul)
      acc[s,h]   += w[e] * tanh(z)             (ACT tanh+scale, DVE add)
  - acc chunks stored straight to DRAM in natural [S,H] layout.
All expert matmuls bf16 with fp32 PSUM accumulation.
"""

import sys

for _p in ("/opt/trn_rl_repo", "/root/.axon_site/_ro/trn_rl_repo"):
    if _p not in sys.path:
        sys.path.insert(0, _p)

import ml_dtypes
import numpy as np

import concourse.bacc as bacc
import concourse.bass as bass
import concourse.mybir as mybir
import concourse.tile as tile
from concourse.bass_utils import run_bass_kernel_spmd

BF16 = ml_dtypes.bfloat16
F32 = mybir.dt.float32
BF = mybir.dt.bfloat16
AF = mybir.ActivationFunctionType
ALU = mybir.AluOpType

B, S, H, M, E = 8, 2048, 2048, 256, 8
Hh = H // 2
CHUNK = 512
NCHUNK = S // CHUNK          # 4
NST = CHUNK // 128           # 4 s-subtiles per chunk
NHT = H // 512               # 4 output h tiles (512 wide)
NFT = Hh // 128              # 8 L1 output f tiles
NKH = H // 128               # 16 k tiles over h

_NC = None


def _softmax_1x8(nc, pool, vec, out, tagp):
    """vec, out: [1, E] f32 sbuf APs. out = softmax(vec) along free dim."""
    mx = pool.tile([1, 1], F32, tag=tagp + "mx", name=tagp + "mx")
    nc.vector.tensor_reduce(mx[:], vec, mybir.AxisListType.X, ALU.max)
    t = pool.tile([1, E], F32, tag=tagp + "t", name=tagp + "t")
    nc.vector.tensor_scalar(t[:], vec, mx[0:1, 0:1], None, ALU.subtract)
    nc.scalar.activation(t[:], t[:], AF.Exp)
    sm = pool.tile([1, 1], F32, tag=tagp + "sm", name=tagp + "sm")
    nc.vector.tensor_reduce(sm[:], t[:], mybir.AxisListType.X, ALU.add)
    rs = pool.tile([1, 1], F32, tag=tagp + "rs", name=tagp + "rs")
    nc.vector.reciprocal(rs[:], sm[:])
    nc.vector.tensor_scalar(out, t[:], rs[0:1, 0:1], None, ALU.mult)


def build():
    nc = bacc.Bacc("TRN2", target_bir_lowering=False, debug=False, num_devices=B)

    x_d = nc.dram_tensor("x", [S, H], BF, kind="ExternalInput")
    w1_d = nc.dram_tensor("W1", [E, H, Hh], BF, kind="ExternalInput")
    w2_d = nc.dram_tensor("W2", [E, Hh, H], BF, kind="ExternalInput")
    b1_d = nc.dram_tensor("b1", [E, Hh], F32, kind="ExternalInput")
    b2_d = nc.dram_tensor("b2", [E, H], BF, kind="ExternalInput")
    wm1_d = nc.dram_tensor("Wm1", [H, M], BF, kind="ExternalInput")
    bm1_d = nc.dram_tensor("bm1", [M], F32, kind="ExternalInput")
    wm2_d = nc.dram_tensor("Wm2", [M, M], BF, kind="ExternalInput")
    bm2_d = nc.dram_tensor("bm2", [M], F32, kind="ExternalInput")
    wm3_d = nc.dram_tensor("Wm3", [M, E], BF, kind="ExternalInput")
    bm3_d = nc.dram_tensor("bm3", [E], F32, kind="ExternalInput")
    eff_d = nc.dram_tensor("eff", [E], F32, kind="ExternalInput")
    out_d = nc.dram_tensor("out", [S, H], F32, kind="ExternalOutput")

    with tile.TileContext(nc) as tc:
        with (
            tc.tile_pool(name="persist", bufs=1) as pp,
            tc.tile_pool(name="router", bufs=1) as rp,
            tc.tile_pool(name="router_xt", bufs=4) as rxp,
            tc.tile_pool(name="xt", bufs=3) as xtp,
            tc.tile_pool(name="w1", bufs=1) as w1p,
            tc.tile_pool(name="w2", bufs=1) as w2p,
            tc.tile_pool(name="bias", bufs=1) as bp,
            tc.tile_pool(name="he", bufs=2) as hep,
            tc.tile_pool(name="acc", bufs=1) as accp,
            tc.tile_pool(name="ye", bufs=3) as yep,
            tc.tile_pool(name="ys", bufs=3) as ysp,
            tc.tile_pool(name="ps1", bufs=2, space=bass.MemorySpace.PSUM) as ps1p,
            tc.tile_pool(name="ps2", bufs=4, space=bass.MemorySpace.PSUM) as ps2p,
            tc.tile_pool(name="rps", bufs=1, space=bass.MemorySpace.PSUM) as rpsp,
        ):
            wbc = pp.tile([128, E], F32)       # router weights, bcast to 128 parts
            ones_bf = pp.tile([1, 128], BF)    # ones row for bias matmuls
            nc.vector.memset(ones_bf[:], 1.0)
            pooled_f = pp.tile([128, NKH], F32)
            nc.vector.memset(pooled_f[:], 0.0)

            def pool_reduce(src, ht, tmp_name):
                r = rxp.tile([128, 1], F32, tag="rred", name=tmp_name)
                nc.vector.tensor_reduce(r[:], src, mybir.AxisListType.X, ALU.add)
                nc.vector.tensor_tensor(
                    pooled_f[:, ht:ht + 1], pooled_f[:, ht:ht + 1], r[:], ALU.add
                )

            # Main-loop xT tiles for chunks 0,1 — hoisted so the router can
            # pool from them before the expert loop starts reading wbc.
            xt_pre = {}
            for ck in (0, 1):
                xt = xtp.tile([128, NKH, CHUNK], BF, tag="xt", name=f"xt{ck}")
                for ht in range(NKH):
                    nc.sync.dma_start_transpose(
                        xt[:, ht, :],
                        x_d[ck * CHUNK:(ck + 1) * CHUNK, ht * 128:(ht + 1) * 128],
                    )
                xt_pre[ck] = xt
            # Router pre-pass transposes for chunks 2,3 (transient tiles).
            pre_tiles = []
            for ck in (2, 3):
                for ht in range(NKH):
                    t = rxp.tile([128, CHUNK], BF, tag="rxt", name=f"rxt{ck}_{ht}")
                    nc.sync.dma_start_transpose(
                        t[:], x_d[ck * CHUNK:(ck + 1) * CHUNK, ht * 128:(ht + 1) * 128]
                    )
                    pre_tiles.append((t, ht))

            def emit_router_tail():
                """Everything after pooled_f is complete: scale, MLP, softmaxes,
                broadcast of w. Expert matmuls don't depend on any of this."""
                pooled = rp.tile([128, NKH], BF)
                nc.vector.tensor_scalar(pooled[:], pooled_f[:], 1.0 / S, None, ALU.mult)

                wm1 = rp.tile([128, NKH, M], BF)
                nc.gpsimd.dma_start(wm1[:], wm1_d[:].rearrange("(t p) f -> p t f", p=128))
                bm1 = rp.tile([128, 2], F32)
                nc.gpsimd.dma_start(bm1[:], bm1_d[:].rearrange("(t p) -> p t", p=128))
                wm2 = rp.tile([128, 2, M], BF)
                nc.gpsimd.dma_start(wm2[:], wm2_d[:].rearrange("(t p) f -> p t f", p=128))
                bm2 = rp.tile([128, 2], F32)
                nc.gpsimd.dma_start(bm2[:], bm2_d[:].rearrange("(t p) -> p t", p=128))
                wm3 = rp.tile([128, 2, E], BF)
                nc.gpsimd.dma_start(wm3[:], wm3_d[:].rearrange("(t p) f -> p t f", p=128))
                bm3 = rp.tile([1, E], F32)
                nc.gpsimd.dma_start(bm3[:], bm3_d[:].rearrange("(a e) -> a e", a=1))
                eff = rp.tile([1, E], F32)
                nc.gpsimd.dma_start(eff[:], eff_d[:].rearrange("(a e) -> a e", a=1))
                ones_f = rp.tile([1, 128], F32)
                nc.vector.memset(ones_f[:], 1.0)
                ones_b1 = rp.tile([1, 1], BF)
                nc.vector.memset(ones_b1[:], 1.0)

                h1t = rp.tile([128, 2], BF)
                for ft in range(2):
                    ps = rpsp.tile([128, E], F32, tag="rps", name=f"rps1_{ft}")
                    for kt in range(NKH):
                        nc.tensor.matmul(
                            ps[:, 0:1],
                            wm1[:, kt, ft * 128:(ft + 1) * 128],
                            pooled[:, kt:kt + 1],
                            start=(kt == 0), stop=(kt == NKH - 1),
                        )
                    nc.scalar.activation(h1t[:, ft:ft + 1], ps[:, 0:1], AF.Relu,
                                         bias=bm1[:, ft:ft + 1])
                h2t = rp.tile([128, 2], BF)
                for ft in range(2):
                    ps = rpsp.tile([128, E], F32, tag="rps", name=f"rps2_{ft}")
                    for kt in range(2):
                        nc.tensor.matmul(
                            ps[:, 0:1],
                            wm2[:, kt, ft * 128:(ft + 1) * 128],
                            h2t_src(h1t, kt),
                            start=(kt == 0), stop=(kt == 1),
                        )
                    nc.scalar.activation(h2t[:, ft:ft + 1], ps[:, 0:1], AF.Relu,
                                         bias=bm2[:, ft:ft + 1])
                psl = rpsp.tile([128, E], F32, tag="rps", name="rpsl")
                for kt in range(2):
                    nc.tensor.matmul(
                        psl[0:1, :], h2t[:, kt:kt + 1], wm3[:, kt, :],
                        start=(kt == 0), stop=False,
                    )
                nc.tensor.matmul(
                    psl[0:1, :], ones_b1[0:1, 0:1], bm3_bf(bm3), start=False, stop=True
                )
                logits = rp.tile([1, E], F32)
                nc.vector.tensor_copy(logits[:], psl[0:1, :])

                probs = rp.tile([1, E], F32)
                _softmax_1x8(nc, rp, logits[:], probs[:], "sm1")
                wpre = rp.tile([1, E], F32)
                nc.vector.tensor_tensor(wpre[:], probs[:], eff[:], ALU.mult)
                wrow = rp.tile([1, E], F32)
                _softmax_1x8(nc, rp, wpre[:], wrow[:], "sm2")

                psw = rpsp.tile([128, E], F32, tag="rps", name="rpsw")
                nc.tensor.matmul(psw[:], ones_f[0:1, :], wrow[0:1, :], start=True, stop=True)
                nc.vector.tensor_copy(wbc[:], psw[:])

            def h2t_src(h1t, kt):
                return h1t[:, kt:kt + 1]

            _bm3bf = {}

            def bm3_bf(bm3):
                if "t" not in _bm3bf:
                    t = rp.tile([1, E], BF)
                    nc.vector.tensor_copy(t[:], bm3[:])
                    _bm3bf["t"] = t
                return _bm3bf["t"][0:1, :]

            # Router pooling: chunks 0,1 from the hoisted main tiles, 2,3
            # from the pre-pass tiles; then the full router tail. All before
            # any expert combine reads wbc.
            for ck in (0, 1):
                for ht in range(NKH):
                    pool_reduce(xt_pre[ck][:, ht, :], ht, f"rr{ck}_{ht}")
            for t, ht in pre_tiles:
                pool_reduce(t[:], ht, f"rp_{t.name}")
            emit_router_tail()

            # ---------------- experts ----------------
            for ck in range(NCHUNK):
                if ck in xt_pre:
                    xt = xt_pre[ck]
                else:
                    xt = xtp.tile([128, NKH, CHUNK], BF, tag="xt", name=f"xt{ck}")
                    for ht in range(NKH):
                        nc.sync.dma_start_transpose(
                            xt[:, ht, :],
                            x_d[ck * CHUNK:(ck + 1) * CHUNK, ht * 128:(ht + 1) * 128],
                        )

                acc_tiles = [
                    accp.tile([128, H], F32, tag=f"acc{st}", name=f"acc{ck}_{st}")
                    for st in range(NST)
                ]
                for e in range(E):
                    w1h = []
                    for half in range(2):
                        t = w1p.tile([128, NKH // 2, Hh], BF, tag=f"w1h{half}",
                                     name=f"w1_{ck}_{e}_{half}")
                        nc.scalar.dma_start(
                            t[:],
                            w1_d[e, half * 1024:(half + 1) * 1024, :]
                            .rearrange("(t p) f -> p t f", p=128),
                        )
                        w1h.append(t)
                    w2 = w2p.tile([128, NFT, H], BF, tag="w2", name=f"w2_{ck}_{e}")
                    nc.scalar.dma_start(w2[:], w2_d[e].rearrange("(t p) f -> p t f", p=128))
                    b1t = bp.tile([128, NFT], F32, tag="b1", name=f"b1_{ck}_{e}")
                    nc.gpsimd.dma_start(b1t[:], b1_d[e].rearrange("(t p) -> p t", p=128))
                    b2t = bp.tile([1, H], BF, tag="b2", name=f"b2_{ck}_{e}")
                    nc.gpsimd.dma_start(b2t[:], b2_d[e:e + 1, :])

                    he = hep.tile([128, NFT, CHUNK], BF, tag="he", name=f"he_{ck}_{e}")
                    for ft in range(NFT):
                        ps = ps1p.tile([128, CHUNK], F32, tag="ps1", name=f"ps1_{ck}_{e}_{ft}")
                        for kt in range(NKH):
                            nc.tensor.matmul(
                                ps[:],
                                w1h[kt // 8][:, kt % 8, ft * 128:(ft + 1) * 128],
                                xt[:, kt, :],
                                start=(kt == 0), stop=(kt == NKH - 1),
                            )
                        nc.scalar.activation(
                            he[:, ft, :], ps[:], AF.Relu, bias=b1t[:, ft:ft + 1],
                        )
                    for st in range(NST):
                        for ht in range(NHT):
                            ps2 = ps2p.tile([128, 512], F32, tag="ps2",
                                            name=f"ps2_{ck}_{e}_{st}_{ht}")
                            for fk in range(NFT):
                                nc.tensor.matmul(
                                    ps2[:],
                                    he[:, fk, st * 128:(st + 1) * 128],
                                    w2[:, fk, ht * 512:(ht + 1) * 512],
                                    start=(fk == 0), stop=False,
                                )
                            nc.tensor.matmul(
                                ps2[:], ones_bf[0:1, :], b2t[0:1, ht * 512:(ht + 1) * 512],
                                start=False, stop=True,
                            )
                            ye = yep.tile([128, 512], F32, tag="ye", name=f"ye_{ck}_{e}_{st}_{ht}")
                            nc.scalar.activation(ye[:], ps2[:], AF.Tanh)
                            accs = acc_tiles[st][:, ht * 512:(ht + 1) * 512]
                            if e == 0:
                                nc.vector.tensor_scalar(
                                    accs, ye[:], wbc[:, 0:1], None, ALU.mult
                                )
                            else:
                                ys = ysp.tile([128, 512], F32, tag="ys",
                                              name=f"ys_{ck}_{e}_{st}_{ht}")
                                nc.scalar.mul(ys[:], ye[:], wbc[:, e:e + 1])
                                nc.vector.tensor_tensor(accs, accs, ys[:], ALU.add)
                        if e == E - 1:
                            r0 = ck * CHUNK + st * 128
                            nc.gpsimd.dma_start(out_d[r0:r0 + 128, :], acc_tiles[st][:])

    nc.compile()
    return nc


def _get_nc():
    global _NC
    if _NC is None:
        _NC = build()
    return _NC


def prep_in_maps(inputs):
    x = np.asarray(inputs["x"], np.float32)
    xbf = x.astype(BF16)
    shared = {
        "W1": np.asarray(inputs["W1"], np.float32).astype(BF16),
        "W2": np.asarray(inputs["W2"], np.float32).astype(BF16),
        "b1": np.asarray(inputs["b1"], np.float32),
        "b2": np.asarray(inputs["b2"], np.float32).astype(BF16),
        "Wm1": np.asarray(inputs["Wm1"], np.float32).astype(BF16),
        "bm1": np.asarray(inputs["bm1"], np.float32),
        "Wm2": np.asarray(inputs["Wm2"], np.float32).astype(BF16),
        "bm2": np.asarray(inputs["bm2"], np.float32),
        "Wm3": np.asarray(inputs["Wm3"], np.float32).astype(BF16),
        "bm3": np.asarray(inputs["bm3"], np.float32),
        "eff": np.asarray(inputs["eff"], np.float32),
    }
    return [dict(shared, x=xbf[b]) for b in range(B)]


def kernel(**inputs):
    nc = _get_nc()
    in_maps = prep_in_maps(inputs)
    res = run_bass_kernel_spmd(nc, in_maps, core_ids=list(range(B)))
    return np.stack([r["out"] for r in res.results])


if __name__ == "__main__":
    rng = np.random.default_rng(0)
    s = 0.02
    ins = {
        "x": rng.standard_normal((B, S, H), dtype=np.float32),
        "Wm1": rng.standard_normal((H, M), dtype=np.float32) * s,
        "bm1": np.zeros(M, np.float32),
        "Wm2": rng.standard_normal((M, M), dtype=np.float32) * s,
        "bm2": np.zeros(M, np.float32),
        "Wm3": rng.standard_normal((M, E), dtype=np.float32) * s,
        "bm3": np.zeros(E, np.float32),
        "W1": rng.standard_normal((E, H, Hh), dtype=np.float32) * s,
        "b1": np.zeros((E, Hh), np.float32),
        "W2": rng.standard_normal((E, Hh, H), dtype=np.float32) * s,
        "b2": np.zeros((E, H), np.float32),
        "eff": np.ones(E, np.float32),
    }
    out = kernel(**ins)
    print("out", out.shape, out.dtype, float(np.abs(out).mean()))
